# revision 12
# baseline (speedup 1.0000x reference)
"""Trainium2 Bass kernel for DetCenterDense: shared 3x3 conv + ReLU + four 1x1
head convs (cls/box/dir/scr, sigmoid on scr), output concatenated on channels.

Full inputs in / full output out. Sharding: 8 cores = batch(4) x H-halves(2).
Each core computes a [20, 256, 512] output shard from a [128, 258, 512]
haloed input shard.

Per-core compute: the 3x3 conv is 9 shifted 1x1 convs accumulated in PSUM.
Output rows are processed in pairs packed into one PSUM tile [128, 512]
(partitions 0:64 = row y, 64:128 = row y+1). ALL matmuls run in (128,64)
column-tiled mode as concurrent T0/T1 pairs (tile_position auto-derived from
out.base_partition), which avoids PE mode-switch drains entirely:
  - middle rows b,c: weight blocks split into 64-col halves, T0/T1 stream the
    same rhs concurrently -> 6 slots per pair
  - edge rows a,d: T0 (row a -> psum[0:64]) and T1 (row d -> psum[64:128])
    stream different rhs concurrently -> 3 slots
  - head 1x1 convs: two K=128 M=64 matmuls (zero-padded block weights), T0
    computes row y's 20 head channels, T1 row y+1's -> 1 slot
Head channel order is (scr, cls, box, dir) so the sigmoid rows sit at
partitions 0:4 / 64:68. Output staged in [128, GRP*512] SBUF groups and
written with 4 large strided DMA descriptors per group.

A run of dependency-free warm-up matmuls on a scratch PSUM bank keeps the PE
HAM activity monitor busy during the ~9us DMA startup window so the real
matmul stream starts at the warm 2.4 GHz clock.
"""

import numpy as np

HS = 256          # output rows per core shard
HALO = HS + 2     # input rows per core shard (1-row halo each side)
W = 512
GRP = 8           # row-pairs per output staging group
NWARM = 38        # warm-up matmuls issued before the real stream

# chunk table: tiny leading chunks so the first matmuls start early
CHUNK_STARTS = [0, 2, 4] + [8 + 8 * c for c in range(31)] + [256]
CHUNK_ROWS = [2, 2, 4] + [8] * 31 + [2]
NCHUNK = len(CHUNK_STARTS)
ROW2CHUNK = []
for _c, (_s, _r) in enumerate(zip(CHUNK_STARTS, CHUNK_ROWS)):
    ROW2CHUNK += [(_c, _j) for _j in range(_r)]

_NC_CACHE = {}


def _build_nc():
    from contextlib import ExitStack

    import concourse.mybir as mybir
    import concourse.tile as tile
    from concourse import bacc

    f32 = mybir.dt.float32
    bf16 = mybir.dt.bfloat16
    Sigmoid = mybir.ActivationFunctionType.Sigmoid

    nc = bacc.Bacc("TRN2", target_bir_lowering=False, debug=False, num_devices=8)
    x_d = nc.dram_tensor("x", [128, HALO * W], bf16, kind="ExternalInput").ap()
    wf_d = nc.dram_tensor("wfull", [128, 6 * 128], bf16, kind="ExternalInput").ap()
    wh_d = nc.dram_tensor("whalf", [128, 6 * 64], bf16, kind="ExternalInput").ap()
    whd_d = nc.dram_tensor("wheads", [128, 128], bf16, kind="ExternalInput").ap()
    b_d = nc.dram_tensor("b128", [128, 1], f32, kind="ExternalInput").ap()
    out_d = nc.dram_tensor("out", [20, HS * W], f32, kind="ExternalOutput").ap()

    with ExitStack() as ctx:
        tc = ctx.enter_context(tile.TileContext(nc))
        wpool = ctx.enter_context(tc.tile_pool(name="w", bufs=1))
        bfpool = ctx.enter_context(tc.tile_pool(name="xbf", bufs=6))
        xrpool = ctx.enter_context(tc.tile_pool(name="xr", bufs=3))
        spool = ctx.enter_context(tc.tile_pool(name="st", bufs=2))
        ppool = ctx.enter_context(tc.tile_pool(name="pp", bufs=3, space="PSUM"))
        hpool = ctx.enter_context(tc.tile_pool(name="hp", bufs=2, space="PSUM"))
        jpool = ctx.enter_context(tc.tile_pool(name="jp", bufs=1, space="PSUM"))

        # HAM warm-up: dependency-free matmuls on the preamble const tiles
        # (memset at t~0) keep the PE activity monitor busy through the DMA
        # startup window so the real stream begins at the warm 2.4 GHz clock.
        jP = jpool.tile([128, W], f32)
        cw = nc.const_aps.tensor(1.0, (128, 64), bf16)
        cb = nc.const_aps.tensor(1.0, (128, W), bf16)
        for _ in range(NWARM):
            nc.tensor.matmul(jP[0:64, :], cw, cb, start=True, stop=True)

        chunks = [None] * NCHUNK

        def load_chunk(c):
            r0, rows = CHUNK_STARTS[c], CHUNK_ROWS[c]
            n = rows * W
            if c < 3:  # small persistent leading chunks
                xb = wpool.tile([128, n], bf16, name=f"xb{c}")
            else:
                xb = bfpool.tile([128, 8 * W], bf16, tag="xb")
            nc.sync.dma_start(xb[:, 0:n], x_d[:, r0 * W : r0 * W + n])
            chunks[c] = xb

        # all loads on one queue, ordered by first use: wf -> rows 0:4 ->
        # wh -> rows 4:8 -> whd -> bias; the prefetch loop continues from 3
        wf = wpool.tile([128, 6 * 128], bf16)
        nc.sync.dma_start(wf[:], wf_d[:])
        load_chunk(0)
        load_chunk(1)
        wh = wpool.tile([128, 6 * 64], bf16)
        nc.sync.dma_start(wh[:], wh_d[:])
        load_chunk(2)
        whd = wpool.tile([128, 128], bf16)
        nc.sync.dma_start(whd[:], whd_d[:])
        bt = wpool.tile([128, 1], f32)
        nc.sync.dma_start(bt[:], b_d[:])

        # per-tap column windows: out[:, so0:so1] += W_kx^T @ in[:, si0:si1]
        CUTS = {0: (0, 511, 1, 512), 1: (0, 512, 0, 512), 2: (1, 512, 0, 511)}

        def row_slice(j, si0, si1):
            c, o = ROW2CHUNK[j]
            return chunks[c][:, o * W + si0 : o * W + si1]

        loaded = 3
        AHEAD = 16  # prefetch horizon in input rows
        S = [None]  # current staging tile

        def flush(g, j0, j1):
            St = S[0]
            y0 = 2 * GRP * g + 2 * j0
            jn = j1 - j0
            ov = out_d[:, y0 * W : (y0 + 2 * jn) * W].rearrange(
                "q (j e w) -> q j e w", j=jn, w=W
            )  # [20, jn, 2, 512]
            csl = slice(j0 * W, j1 * W)
            Se = St[4:20, csl].rearrange("p (j w) -> p j w", w=W)
            So = St[68:84, csl].rearrange("p (j w) -> p j w", w=W)
            Sse = St[0:4, csl].rearrange("p (j w) -> p j w", w=W)
            Sso = St[64:68, csl].rearrange("p (j w) -> p j w", w=W)
            nc.gpsimd.dma_start(ov[0:16, :, 0, :], Se)
            nc.sync.dma_start(ov[0:16, :, 1, :], So)
            nc.gpsimd.dma_start(ov[16:20, :, 0, :], Sse)
            nc.scalar.dma_start(ov[16:20, :, 1, :], Sso)

        def head_and_emit(q, xr_q):
            # head matmuls for pair q (rows 2q, 2q+1): T0 -> row y's 20 head
            # channels at hP[0:20], T1 -> row y+1's at hP[64:84]
            hP = hpool.tile([128, W], f32, tag="hp")
            nc.tensor.matmul(hP[0:64, :], whd[:, 0:64], xr_q[:, :], start=True, stop=True)
            nc.tensor.matmul(hP[64:128, :], whd[:, 64:128], xr_q[:, :], start=True, stop=True)

            i = q % GRP
            if i == 0:
                snew = spool.tile([128, GRP * W], f32, tag="S")
                S[0] = snew
            St = S[0]
            sl = slice(i * W, (i + 1) * W)
            # one DVE add covers all head rows (20:64 are zeros; PSUM reads
            # must start 32-aligned); ACT then overwrites the sigmoid rows
            nc.vector.tensor_scalar_add(St[0:84, sl], hP[0:84, :], bt[0:84])
            nc.scalar.activation(St[0:4, sl], hP[0:4, :], Sigmoid, bias=bt[0:4])
            nc.scalar.activation(St[64:68, sl], hP[64:68, :], Sigmoid, bias=bt[64:68])

            NP = HS // 2
            if q == NP - GRP + 3:      # first half of the last group, early
                flush(q // GRP, 0, 4)
            elif q == NP - 1:          # second half of the last group
                flush(q // GRP, 4, GRP)
            elif i == GRP - 1:
                flush(q // GRP, 0, GRP)

        pending = None  # (q, xr) of the previous pair, head not yet emitted

        for p in range(HS // 2):
            while loaded < NCHUNK and CHUNK_STARTS[loaded] <= 2 * p + 3 + AHEAD:
                load_chunk(loaded)
                loaded += 1

            P = ppool.tile([128, W], f32, tag="pp")
            a, b, c, d = 2 * p, 2 * p + 1, 2 * p + 2, 2 * p + 3
            firstT = [True, True]
            # middle rows b ([W1|W0]) then c ([W2|W1]); each 128-col weight
            # block split into T0/T1 halves streaming the same rhs
            for t_idx, j in ((0, b), (1, c)):
                for kx in (1, 0, 2):
                    si0, si1, so0, so1 = CUTS[kx]
                    blk = 2 * kx + t_idx
                    rs = row_slice(j, si0, si1)
                    for half in (0, 1):
                        nc.tensor.matmul(
                            P[64 * half : 64 * half + 64, so0:so1],
                            wf[:, blk * 128 + 64 * half : blk * 128 + 64 * half + 64],
                            rs,
                            start=firstT[half],
                            stop=False,
                        )
                        firstT[half] = False
            # edge rows: a (W0 -> psum[0:64] on T0), d (W2 -> psum[64:128] on T1)
            for kx in (1, 0, 2):
                si0, si1, so0, so1 = CUTS[kx]
                last = kx == 2
                nc.tensor.matmul(
                    P[0:64, so0:so1],
                    wh[:, (2 * kx) * 64 : (2 * kx + 1) * 64],
                    row_slice(a, si0, si1),
                    start=False,
                    stop=last,
                )
                nc.tensor.matmul(
                    P[64:128, so0:so1],
                    wh[:, (2 * kx + 1) * 64 : (2 * kx + 2) * 64],
                    row_slice(d, si0, si1),
                    start=False,
                    stop=last,
                )

            # relu first on the DVE queue: the next pair's head matmuls need it
            xr = xrpool.tile([128, W], bf16, tag="xr")
            nc.vector.tensor_scalar_max(xr[:], P[:], 0.0)

            if pending is not None:
                head_and_emit(*pending)
            pending = (p, xr)

        head_and_emit(*pending)

    nc.compile()
    return nc


def _get_nc():
    if "nc" not in _NC_CACHE:
        _NC_CACHE["nc"] = _build_nc()
    return _NC_CACHE["nc"]


def _pack_weights(w_shared, w_cls, b_cls, w_box, b_box, w_dir, b_dir, w_scr, b_scr):
    import ml_dtypes

    Wt = np.ascontiguousarray(w_shared, np.float32).transpose(1, 0, 2, 3)  # [128,64,3,3]
    wfull = np.zeros((128, 6, 128), np.float32)
    whalf = np.zeros((128, 6, 64), np.float32)
    for kx in range(3):
        wfull[:, 2 * kx + 0, 0:64] = Wt[:, :, 1, kx]
        wfull[:, 2 * kx + 0, 64:128] = Wt[:, :, 0, kx]
        wfull[:, 2 * kx + 1, 0:64] = Wt[:, :, 2, kx]
        wfull[:, 2 * kx + 1, 64:128] = Wt[:, :, 1, kx]
        whalf[:, 2 * kx + 0] = Wt[:, :, 0, kx]
        whalf[:, 2 * kx + 1] = Wt[:, :, 2, kx]
    wfull = np.ascontiguousarray(wfull.reshape(128, 768)).astype(ml_dtypes.bfloat16)
    whalf = np.ascontiguousarray(whalf.reshape(128, 384)).astype(ml_dtypes.bfloat16)

    # head channel order (scr, cls, box, dir): sigmoid rows at partitions 0:4
    Wh = np.concatenate([w_scr, w_cls, w_box, w_dir], 0)[:, :, 0, 0].astype(np.float32)  # [20,64]
    bh = np.concatenate([b_scr, b_cls, b_box, b_dir], 0).astype(np.float32)  # [20]
    # T0 block (cols 0:64): K rows 0:64 (= row y channels); T1 block (cols
    # 64:128): K rows 64:128 (= row y+1 channels); head outputs in cols 0:20
    wheads = np.zeros((128, 128), np.float32)
    wheads[0:64, 0:20] = Wh.T
    wheads[64:128, 64:84] = Wh.T
    wheads = wheads.astype(ml_dtypes.bfloat16)
    b128 = np.zeros((128, 1), np.float32)
    b128[0:20, 0] = bh
    b128[64:84, 0] = bh
    return wfull, whalf, wheads, b128


def _make_in_maps(feature, packed):
    import ml_dtypes

    wfull, whalf, wheads, b128 = packed
    in_maps = []
    for core in range(8):
        bi, half = core // 2, core % 2
        r0 = half * HS
        xs = np.zeros((128, HALO, W), ml_dtypes.bfloat16)
        lo, hi = r0 - 1, r0 + HS + 1
        slo, shi = max(lo, 0), min(hi, 512)
        xs[:, slo - lo : HALO - (hi - shi), :] = feature[bi, :, slo:shi, :].astype(
            ml_dtypes.bfloat16
        )
        in_maps.append(
            {
                "x": xs.reshape(128, HALO * W),
                "wfull": wfull,
                "whalf": whalf,
                "wheads": wheads,
                "b128": b128,
            }
        )
    return in_maps


def _run(inputs, trace=False):
    from concourse.bass_utils import run_bass_kernel_spmd

    feature = np.ascontiguousarray(inputs["feature"], np.float32)  # [4,128,512,512]
    B, Cin, H, Wd = feature.shape
    assert (B, Cin, H, Wd) == (4, 128, 512, 512)

    packed = _pack_weights(
        np.asarray(inputs["w_shared"]),
        np.asarray(inputs["w_cls"]), np.asarray(inputs["b_cls"]),
        np.asarray(inputs["w_box"]), np.asarray(inputs["b_box"]),
        np.asarray(inputs["w_dir"]), np.asarray(inputs["b_dir"]),
        np.asarray(inputs["w_scr"]), np.asarray(inputs["b_scr"]),
    )
    in_maps = _make_in_maps(feature, packed)
    nc = _get_nc()
    res = run_bass_kernel_spmd(nc, in_maps, core_ids=list(range(8)), trace=trace)

    out = np.empty((4, 20, 512, 512), np.float32)
    for core in range(8):
        bi, half = core // 2, core % 2
        out[bi, :, half * HS : (half + 1) * HS, :] = res.results[core]["out"].reshape(
            20, HS, W
        )
    return out, res


def kernel(**inputs):
    out, _ = _run(inputs, trace=False)
    return out


def run_traced(**inputs):
    """Like kernel(), but returns (out, BassKernelResults) with a profile trace."""
    return _run(inputs, trace=True)


# revision 13
# speedup vs baseline: 1.0126x; 1.0126x over previous
"""Trainium2 Bass kernel for DetCenterDense: shared 3x3 conv + ReLU + four 1x1
head convs (cls/box/dir/scr, sigmoid on scr), output concatenated on channels.

Full inputs in / full output out. Sharding: 8 cores = batch(4) x H-halves(2).
Each core computes a [20, 256, 512] output shard from a [128, 258, 512]
haloed input shard.

Per-core compute: the 3x3 conv is 9 shifted 1x1 convs accumulated in PSUM.
Output rows are processed in pairs packed into one PSUM tile [128, 512]
(partitions 0:64 = row y, 64:128 = row y+1). ALL matmuls run in (128,64)
column-tiled mode as concurrent T0/T1 pairs (tile_position auto-derived from
out.base_partition), which avoids PE mode-switch drains entirely:
  - middle rows b,c: weight blocks split into 64-col halves, T0/T1 stream the
    same rhs concurrently -> 6 slots per pair
  - edge rows a,d: T0 (row a -> psum[0:64]) and T1 (row d -> psum[64:128])
    stream different rhs concurrently -> 3 slots
  - head 1x1 convs: two K=128 M=64 matmuls (zero-padded block weights), T0
    computes row y's 20 head channels, T1 row y+1's -> 1 slot
Head channel order is (scr, cls, box, dir) so the sigmoid rows sit at
partitions 0:4 / 64:68. Output staged in [128, GRP*512] SBUF groups and
written with 4 large strided DMA descriptors per group.

A run of dependency-free warm-up matmuls on a scratch PSUM bank keeps the PE
HAM activity monitor busy during the ~9us DMA startup window so the real
matmul stream starts at the warm 2.4 GHz clock.
"""

import numpy as np

HS = 256          # output rows per core shard
HALO = HS + 2     # input rows per core shard (1-row halo each side)
W = 512
GRP = 8           # row-pairs per output staging group
NWARM = 11        # warm-up matmuls issued before the real stream

# chunk table: tiny leading chunks so the first matmuls start early
CHUNK_STARTS = [0, 2, 4] + [8 + 8 * c for c in range(31)] + [256]
CHUNK_ROWS = [2, 2, 4] + [8] * 31 + [2]
NCHUNK = len(CHUNK_STARTS)
ROW2CHUNK = []
for _c, (_s, _r) in enumerate(zip(CHUNK_STARTS, CHUNK_ROWS)):
    ROW2CHUNK += [(_c, _j) for _j in range(_r)]

_NC_CACHE = {}


def _build_nc():
    from contextlib import ExitStack

    import concourse.mybir as mybir
    import concourse.tile as tile
    from concourse import bacc

    f32 = mybir.dt.float32
    bf16 = mybir.dt.bfloat16
    Sigmoid = mybir.ActivationFunctionType.Sigmoid

    nc = bacc.Bacc("TRN2", target_bir_lowering=False, debug=False, num_devices=8)
    x_d = nc.dram_tensor("x", [128, HALO * W], bf16, kind="ExternalInput").ap()
    wf_d = nc.dram_tensor("wfull", [128, 6 * 128], bf16, kind="ExternalInput").ap()
    wh_d = nc.dram_tensor("whalf", [128, 6 * 64], bf16, kind="ExternalInput").ap()
    whd_d = nc.dram_tensor("wheads", [128, 128], bf16, kind="ExternalInput").ap()
    b_d = nc.dram_tensor("b128", [128, 1], f32, kind="ExternalInput").ap()
    out_d = nc.dram_tensor("out", [20, HS * W], f32, kind="ExternalOutput").ap()

    with ExitStack() as ctx:
        tc = ctx.enter_context(tile.TileContext(nc))
        wpool = ctx.enter_context(tc.tile_pool(name="w", bufs=1))
        bfpool = ctx.enter_context(tc.tile_pool(name="xbf", bufs=6))
        xrpool = ctx.enter_context(tc.tile_pool(name="xr", bufs=3))
        spool = ctx.enter_context(tc.tile_pool(name="st", bufs=2))
        ppool = ctx.enter_context(tc.tile_pool(name="pp", bufs=3, space="PSUM"))
        hpool = ctx.enter_context(tc.tile_pool(name="hp", bufs=2, space="PSUM"))
        jpool = ctx.enter_context(tc.tile_pool(name="jp", bufs=1, space="PSUM"))

        # HAM warm-up: dependency-free matmuls on the preamble const tiles
        # (memset at t~0) keep the PE activity monitor busy through the DMA
        # startup window so the real stream begins at the warm 2.4 GHz clock.
        jP = jpool.tile([128, W], f32)
        cw = nc.const_aps.tensor(1.0, (128, 64), bf16)
        cb = nc.const_aps.tensor(1.0, (128, W), bf16)
        for _ in range(NWARM):
            nc.tensor.matmul(jP[0:64, :], cw, cb, start=True, stop=True)

        chunks = [None] * NCHUNK

        def load_chunk(c):
            r0, rows = CHUNK_STARTS[c], CHUNK_ROWS[c]
            n = rows * W
            if c < 3:  # small persistent leading chunks
                xb = wpool.tile([128, n], bf16, name=f"xb{c}")
            else:
                xb = bfpool.tile([128, 8 * W], bf16, tag="xb")
            nc.sync.dma_start(xb[:, 0:n], x_d[:, r0 * W : r0 * W + n])
            chunks[c] = xb

        # all loads on one queue, ordered by first use: wf -> rows 0:4 ->
        # wh -> rows 4:8 -> whd -> bias; the prefetch loop continues from 3
        wf = wpool.tile([128, 6 * 128], bf16)
        nc.sync.dma_start(wf[:], wf_d[:])
        load_chunk(0)
        load_chunk(1)
        wh = wpool.tile([128, 6 * 64], bf16)
        nc.sync.dma_start(wh[:], wh_d[:])
        load_chunk(2)
        whd = wpool.tile([128, 128], bf16)
        nc.sync.dma_start(whd[:], whd_d[:])
        bt = wpool.tile([128, 1], f32)
        nc.sync.dma_start(bt[:], b_d[:])

        # per-tap column windows: out[:, so0:so1] += W_kx^T @ in[:, si0:si1]
        CUTS = {0: (0, 511, 1, 512), 1: (0, 512, 0, 512), 2: (1, 512, 0, 511)}

        def row_slice(j, si0, si1):
            c, o = ROW2CHUNK[j]
            return chunks[c][:, o * W + si0 : o * W + si1]

        loaded = 3
        AHEAD = 24  # prefetch horizon in input rows
        S = [None]  # current staging tile

        def flush(g, j0, j1):
            St = S[0]
            y0 = 2 * GRP * g + 2 * j0
            jn = j1 - j0
            ov = out_d[:, y0 * W : (y0 + 2 * jn) * W].rearrange(
                "q (j e w) -> q j e w", j=jn, w=W
            )  # [20, jn, 2, 512]
            csl = slice(j0 * W, j1 * W)
            Se = St[4:20, csl].rearrange("p (j w) -> p j w", w=W)
            So = St[68:84, csl].rearrange("p (j w) -> p j w", w=W)
            Sse = St[0:4, csl].rearrange("p (j w) -> p j w", w=W)
            Sso = St[64:68, csl].rearrange("p (j w) -> p j w", w=W)
            nc.gpsimd.dma_start(ov[0:16, :, 0, :], Se)
            nc.sync.dma_start(ov[0:16, :, 1, :], So)
            nc.gpsimd.dma_start(ov[16:20, :, 0, :], Sse)
            nc.scalar.dma_start(ov[16:20, :, 1, :], Sso)

        def head_and_emit(q, xr_q):
            # head matmuls for pair q (rows 2q, 2q+1): T0 -> row y's 20 head
            # channels at hP[0:20], T1 -> row y+1's at hP[64:84]
            hP = hpool.tile([128, W], f32, tag="hp")
            nc.tensor.matmul(hP[0:64, :], whd[:, 0:64], xr_q[:, :], start=True, stop=True)
            nc.tensor.matmul(hP[64:128, :], whd[:, 64:128], xr_q[:, :], start=True, stop=True)

            i = q % GRP
            if i == 0:
                snew = spool.tile([128, GRP * W], f32, tag="S")
                S[0] = snew
            St = S[0]
            sl = slice(i * W, (i + 1) * W)
            # one DVE add covers all head rows (20:64 are zeros; PSUM reads
            # must start 32-aligned); ACT then overwrites the sigmoid rows
            nc.vector.tensor_scalar_add(St[0:84, sl], hP[0:84, :], bt[0:84])
            nc.scalar.activation(St[0:4, sl], hP[0:4, :], Sigmoid, bias=bt[0:4])
            nc.scalar.activation(St[64:68, sl], hP[64:68, :], Sigmoid, bias=bt[64:68])

            NP = HS // 2
            if q == NP - GRP + 3:      # first half of the last group, early
                flush(q // GRP, 0, 4)
            elif q == NP - 1:          # second half of the last group
                flush(q // GRP, 4, GRP)
            elif i == GRP - 1:
                flush(q // GRP, 0, GRP)

        pending = None  # (q, xr) of the previous pair, head not yet emitted

        for p in range(HS // 2):
            while loaded < NCHUNK and CHUNK_STARTS[loaded] <= 2 * p + 3 + AHEAD:
                load_chunk(loaded)
                loaded += 1

            P = ppool.tile([128, W], f32, tag="pp")
            a, b, c, d = 2 * p, 2 * p + 1, 2 * p + 2, 2 * p + 3
            firstT = [True, True]
            # middle rows b ([W1|W0]) then c ([W2|W1]); each 128-col weight
            # block split into T0/T1 halves streaming the same rhs
            for t_idx, j in ((0, b), (1, c)):
                for kx in (1, 0, 2):
                    si0, si1, so0, so1 = CUTS[kx]
                    blk = 2 * kx + t_idx
                    rs = row_slice(j, si0, si1)
                    for half in (0, 1):
                        nc.tensor.matmul(
                            P[64 * half : 64 * half + 64, so0:so1],
                            wf[:, blk * 128 + 64 * half : blk * 128 + 64 * half + 64],
                            rs,
                            start=firstT[half],
                            stop=False,
                        )
                        firstT[half] = False
            # edge rows: a (W0 -> psum[0:64] on T0), d (W2 -> psum[64:128] on T1)
            for kx in (1, 0, 2):
                si0, si1, so0, so1 = CUTS[kx]
                last = kx == 2
                nc.tensor.matmul(
                    P[0:64, so0:so1],
                    wh[:, (2 * kx) * 64 : (2 * kx + 1) * 64],
                    row_slice(a, si0, si1),
                    start=False,
                    stop=last,
                )
                nc.tensor.matmul(
                    P[64:128, so0:so1],
                    wh[:, (2 * kx + 1) * 64 : (2 * kx + 2) * 64],
                    row_slice(d, si0, si1),
                    start=False,
                    stop=last,
                )

            # relu first on the DVE queue: the next pair's head matmuls need it
            xr = xrpool.tile([128, W], bf16, tag="xr")
            nc.vector.tensor_scalar_max(xr[:], P[:], 0.0)

            if pending is not None:
                head_and_emit(*pending)
            pending = (p, xr)

        head_and_emit(*pending)

    nc.compile()
    return nc


def _get_nc():
    if "nc" not in _NC_CACHE:
        _NC_CACHE["nc"] = _build_nc()
    return _NC_CACHE["nc"]


def _pack_weights(w_shared, w_cls, b_cls, w_box, b_box, w_dir, b_dir, w_scr, b_scr):
    import ml_dtypes

    Wt = np.ascontiguousarray(w_shared, np.float32).transpose(1, 0, 2, 3)  # [128,64,3,3]
    wfull = np.zeros((128, 6, 128), np.float32)
    whalf = np.zeros((128, 6, 64), np.float32)
    for kx in range(3):
        wfull[:, 2 * kx + 0, 0:64] = Wt[:, :, 1, kx]
        wfull[:, 2 * kx + 0, 64:128] = Wt[:, :, 0, kx]
        wfull[:, 2 * kx + 1, 0:64] = Wt[:, :, 2, kx]
        wfull[:, 2 * kx + 1, 64:128] = Wt[:, :, 1, kx]
        whalf[:, 2 * kx + 0] = Wt[:, :, 0, kx]
        whalf[:, 2 * kx + 1] = Wt[:, :, 2, kx]
    wfull = np.ascontiguousarray(wfull.reshape(128, 768)).astype(ml_dtypes.bfloat16)
    whalf = np.ascontiguousarray(whalf.reshape(128, 384)).astype(ml_dtypes.bfloat16)

    # head channel order (scr, cls, box, dir): sigmoid rows at partitions 0:4
    Wh = np.concatenate([w_scr, w_cls, w_box, w_dir], 0)[:, :, 0, 0].astype(np.float32)  # [20,64]
    bh = np.concatenate([b_scr, b_cls, b_box, b_dir], 0).astype(np.float32)  # [20]
    # T0 block (cols 0:64): K rows 0:64 (= row y channels); T1 block (cols
    # 64:128): K rows 64:128 (= row y+1 channels); head outputs in cols 0:20
    wheads = np.zeros((128, 128), np.float32)
    wheads[0:64, 0:20] = Wh.T
    wheads[64:128, 64:84] = Wh.T
    wheads = wheads.astype(ml_dtypes.bfloat16)
    b128 = np.zeros((128, 1), np.float32)
    b128[0:20, 0] = bh
    b128[64:84, 0] = bh
    return wfull, whalf, wheads, b128


def _make_in_maps(feature, packed):
    import ml_dtypes

    wfull, whalf, wheads, b128 = packed
    in_maps = []
    for core in range(8):
        bi, half = core // 2, core % 2
        r0 = half * HS
        xs = np.zeros((128, HALO, W), ml_dtypes.bfloat16)
        lo, hi = r0 - 1, r0 + HS + 1
        slo, shi = max(lo, 0), min(hi, 512)
        xs[:, slo - lo : HALO - (hi - shi), :] = feature[bi, :, slo:shi, :].astype(
            ml_dtypes.bfloat16
        )
        in_maps.append(
            {
                "x": xs.reshape(128, HALO * W),
                "wfull": wfull,
                "whalf": whalf,
                "wheads": wheads,
                "b128": b128,
            }
        )
    return in_maps


def _run(inputs, trace=False):
    from concourse.bass_utils import run_bass_kernel_spmd

    feature = np.ascontiguousarray(inputs["feature"], np.float32)  # [4,128,512,512]
    B, Cin, H, Wd = feature.shape
    assert (B, Cin, H, Wd) == (4, 128, 512, 512)

    packed = _pack_weights(
        np.asarray(inputs["w_shared"]),
        np.asarray(inputs["w_cls"]), np.asarray(inputs["b_cls"]),
        np.asarray(inputs["w_box"]), np.asarray(inputs["b_box"]),
        np.asarray(inputs["w_dir"]), np.asarray(inputs["b_dir"]),
        np.asarray(inputs["w_scr"]), np.asarray(inputs["b_scr"]),
    )
    in_maps = _make_in_maps(feature, packed)
    nc = _get_nc()
    res = run_bass_kernel_spmd(nc, in_maps, core_ids=list(range(8)), trace=trace)

    out = np.empty((4, 20, 512, 512), np.float32)
    for core in range(8):
        bi, half = core // 2, core % 2
        out[bi, :, half * HS : (half + 1) * HS, :] = res.results[core]["out"].reshape(
            20, HS, W
        )
    return out, res


def kernel(**inputs):
    out, _ = _run(inputs, trace=False)
    return out


def run_traced(**inputs):
    """Like kernel(), but returns (out, BassKernelResults) with a profile trace."""
    return _run(inputs, trace=True)
